# revision 1
# baseline (speedup 1.0000x reference)
"""Trainium2 Bass kernel for the Critic model (attention-pointer critic).

Math (per batch b, with coords = raw-reshape(static[b]) as [2, N]):
    sh  = enc_w @ coords + enc_b                       [H, N]
    for layer i in 1..3:
        e_i  = ref_wi @ sh + ref_bi                    [H, N]
        q_i  = q_wi @ hy + q_bi                        [H]
        u_i  = v_i . tanh(e_i + q_i)                   [N]
        p_i  = softmax(u_i)
        hy   = e_i @ p_i                               [H]
    out = fc2 @ relu(fc1 @ hy + fc1_b) + fc2_b         [1]

Everything upstream/downstream of the nonlinearities is linear in coords, so
fold on-device:
    W'_i  = ref_wi @ enc_w           [H, 2]
    b'_i  = ref_wi @ enc_b + ref_bi  [H]
    e_i   = W'_i @ coords + b'_i
    hy_i  = W'_i @ z_i + b'_i   where  z_i = coords @ p_i  (a 2-vector!)
    q_{i+1} = (q_w_{i+1} @ W'_i) @ z_i + (q_w_{i+1} @ b'_i + q_b_{i+1})
    fc1 @ hy_3 + fc1_b = (fc1 @ W'_3) @ z_3 + (fc1 @ b'_3 + fc1_b)

So the only O(H*N) work per (batch, layer) is:
    e = W' @ coords        K=2 matmul on PE (fp32r, full rate at N>=256)
    t = tanh(e + qeff)     ACT, per-partition bias  (the bottleneck engine)
    u = v . t              K=H matmul on PE via one-hot-masked v weights,
                           accumulating u into row b of a [32, N] psum tile
and per layer (batched over the 32 local batches, b in partitions):
    softmax over N on DVE/ACT, z = (p*X).sum / sum(p) via fused mul-reduce.

Hardware constraints honored here:
  * PE operand/output base partitions must be 0/32/64 -> coords live as
    [2, group, N] tiles (batch in free dim); u rows land via masked weights.
  * This walrus build allows AT MOST ONE sync wait per instruction struct:
    _split_multi_waits post-processes the scheduled BIR, hoisting extra
    waits onto standalone InstEventSemaphore instructions (engines are
    in-order, so semantics are identical), chunking semaphore range-clears
    to <= 8 sems, and stripping embedded sync from custom DVE ops.
  * Scheduling: every engine's program order is fixed at emission priority,
    so all weight DMAs/folding are emitted lazily next to their consumers
    (layer-1 path first; later layers' prep is emitted mid-loop to fill PE
    slack under the ACT-bound steady state).

Sharding: pure data-parallel, 32 batches per core across 8 cores; all
weights replicated. ACT (tanh) is the roofline at ~200us busy/core.
"""

import sys

if "/opt/trn_rl_repo" not in sys.path:
    sys.path.insert(0, "/opt/trn_rl_repo")

from contextlib import ExitStack

import numpy as np

import concourse.bass as bass
import concourse.tile as tile
from concourse import mybir
from concourse.bass import _add_dep_helper
from concourse.bass_utils import run_bass_kernel_spmd


def _order(after, before):
    """Force `after` to schedule after `before` (same-engine order, no sem)."""
    _add_dep_helper(after.ins, before.ins, sync=False, reason="wait-budget order")

B, N, H = 256, 1000, 256
NCORES = 8
BC = B // NCORES  # batches per core
GB = 8            # batches per coords tile

F32 = mybir.dt.float32
F32R = mybir.dt.float32r
AF = mybir.ActivationFunctionType
ALU = mybir.AluOpType
AX = mybir.AxisListType

# PSUM bank = 2KB = 512 fp32; matmul output must stay within one bank.
NCH = (0, 512, 1000)


def _split_multi_waits(nc):
    """Walrus in this container accepts at most one sync wait per
    instruction struct. Hoist extra waits onto standalone InstEventSemaphore
    instructions inserted just before the owner (engines are in-order, so the
    semantics are identical)."""
    import os
    split_max = int(os.environ.get("SPLIT_MAX", "999999"))
    nsofar = [0]

    def mk_ev(inst, w):
        ev = mybir.InstEventSemaphore(name=nc.get_next_instruction_name())
        ev.engine = inst.engine
        ev.sync_info = mybir.SyncInfo(on_wait=[w], on_update=[])
        ev.debug = mybir.OpDebugInfo(
            op_name=f"splitwait:{inst.name}:{w.ant_name}",
            filename="kernel.py", lineno=1)
        nc.register_instruction(ev)
        return ev

    f = nc.m.functions[0]
    blocks = list(f.blocks)

    # EVENT_SEMAPHORE_RANGE_CLEAR supports at most 8 semaphores per
    # instruction on this walrus; chunk wider ranges.
    for blk in blocks:
        old_insts = blk.instructions
        rewritten = []
        changed = False
        for inst in old_insts:
            if (type(inst).__name__ == "InstISA"
                    and inst.op_name == "EVENT_SEMAPHORE_RANGE_CLEAR"):
                d = dict(inst.ant_dict)
                first, last = d["range_first"], d["range_last"]
                if last - first + 1 > 8:
                    changed = True
                    lo = first
                    while lo <= last:
                        hi = min(lo + 7, last)
                        nb = list(inst.instr)
                        nb[13], nb[14] = lo, hi
                        d2 = dict(d)
                        d2["range_first"], d2["range_last"] = lo, hi
                        ni = mybir.InstISA(
                            name=nc.get_next_instruction_name(),
                            isa_opcode=inst.isa_opcode,
                            engine=inst.engine,
                            instr=nb,
                            op_name=inst.op_name,
                            ins=[], outs=[],
                            ant_dict=d2,
                            verify=inst.verify,
                            ant_isa_is_sequencer_only=inst.ant_isa_is_sequencer_only,
                        )
                        if inst.sync_info is not None and lo == first:
                            ni.sync_info = inst.sync_info
                        nc.register_instruction(ni)
                        rewritten.append(ni)
                        lo = hi + 1
                    continue
            rewritten.append(inst)
        if changed:
            blk.instructions = rewritten

    for bi, blk in enumerate(blocks):
        old = blk.instructions
        if not any(i.sync_info is not None and len(i.sync_info.on_wait) > 1
                   for i in old):
            continue
        new = []
        hoist_prev = []  # evsems that must run before this block is entered
        for idx, inst in enumerate(old):
            si = inst.sync_info
            is_custom = type(inst).__name__ in ("InstReciprocal",)
            if si is not None and is_custom and (si.on_wait or si.on_update):
                # custom-DVE ops lower to fixed-length ISA payloads that
                # cannot carry embedded sync: hoist waits before, updates
                # after (engine is in-order, semantics unchanged).
                for w in si.on_wait:
                    new.append(mk_ev(inst, w))
                posts = list(si.on_update)
                inst.sync_info = mybir.SyncInfo(on_wait=[], on_update=[])
                new.append(inst)
                for u in posts:
                    ev = mybir.InstEventSemaphore(
                        name=nc.get_next_instruction_name())
                    ev.engine = inst.engine
                    ev.sync_info = mybir.SyncInfo(on_wait=[], on_update=[u])
                    ev.debug = mybir.OpDebugInfo(
                        op_name=f"splitupd:{inst.name}",
                        filename="kernel.py", lineno=1)
                    nc.register_instruction(ev)
                    new.append(ev)
                continue
            if si is not None and len(si.on_wait) > 1 and nsofar[0] < split_max:
                nsofar[0] += 1
                waits = list(si.on_wait)
                evs = [mk_ev(inst, w) for w in waits[:-1]]
                if idx == 0 and bi > 0 and type(inst).__name__ == "InstDrain":
                    # barrier-teardown block: walrus rejects extra
                    # instructions before the first drain, so run the waits
                    # at the tail of the previous block instead.
                    hoist_prev.extend(evs)
                else:
                    new.extend(evs)
                inst.sync_info = mybir.SyncInfo(on_wait=[waits[-1]],
                                                on_update=list(si.on_update))
            new.append(inst)
        blk.instructions = new
        if hoist_prev:
            prev = blocks[bi - 1]
            pinsts = prev.instructions
            cut = len(pinsts)
            while cut > 0 and "Branch" in type(pinsts[cut - 1]).__name__:
                cut -= 1
            prev.instructions = pinsts[:cut] + hoist_prev + pinsts[cut:]


def build_nc():
    nc = bass.Bass(trn_type="TRN2", target_bir_lowering=False)

    def din(name, shape):
        return nc.dram_tensor(name, shape, F32, kind="ExternalInput").ap()

    x = din("x", [2 * BC, N])      # per batch: row 2b = coords[0], 2b+1 = coords[1]
    x0 = din("x0", [BC, N])        # coords row 0, b-partition layout
    x1 = din("x1", [BC, N])        # coords row 1
    ident = din("ident", [128, 128])
    i32f = din("i32f", [1, 32 * 32])  # eye(32) flattened
    enc_w = din("enc_w", [H, 2])
    enc_b = din("enc_b", [H])
    mats = {}
    vecs = {}
    for i in (1, 2, 3):
        mats[f"ref_w{i}"] = din(f"ref_w{i}", [H, H])
        vecs[f"ref_b{i}"] = din(f"ref_b{i}", [H])
        if i > 1:
            mats[f"q_w{i}"] = din(f"q_w{i}", [H, H])
        vecs[f"q_b{i}"] = din(f"q_b{i}", [H])
        vecs[f"v{i}"] = din(f"v{i}", [H])
    mats["fc1_w"] = din("fc1_w", [H, H])
    vecs["fc1_b"] = din("fc1_b", [H])
    fc2_w = din("fc2_w", [1, H])
    fc2_b = din("fc2_b", [1])
    out = nc.dram_tensor("out", [BC], F32, kind="ExternalOutput").ap()

    with ExitStack() as ctx:
        tc = ctx.enter_context(tile.TileContext(nc))
        const = ctx.enter_context(tc.tile_pool(name="const", bufs=1))
        tp = ctx.enter_context(tc.tile_pool(name="tp", bufs=8))
        work = ctx.enter_context(tc.tile_pool(name="work", bufs=2))
        scr = ctx.enter_context(tc.tile_pool(name="scr", bufs=2))
        ppool = ctx.enter_context(tc.tile_pool(name="ppool", bufs=3))
        ep = ctx.enter_context(tc.tile_pool(name="ep", bufs=2, space="PSUM"))
        up = ctx.enter_context(tc.tile_pool(name="up", bufs=1, space="PSUM"))
        sp = ctx.enter_context(tc.tile_pool(name="sp", bufs=2, space="PSUM"))

        mm = nc.tensor.matmul
        act = nc.scalar.activation
        dve = nc.vector
        dma = nc.sync.dma_start  # HWDGE: 8 parallel queues (waits get split)

        # ------------- loads: only the layer-1 critical path up front -------
        ids = const.tile([128, 128], F32, tag="ids", name="ids")
        dma(out=ids, in_=ident)
        encs = const.tile([128, 2, 2], F32, tag="encs", name="encs")
        dma(out=encs, in_=enc_w.rearrange("(j p) c -> p j c", p=128))

        vs = {}

        def vsget(key):
            if key not in vs:
                t = const.tile([128, 2], F32, tag=f"v_{key}", name=f"v_{key}")
                dma(out=t, in_=vecs[key].rearrange("(j p) -> p j", p=128))
                vs[key] = t
            return vs[key]

        encb = const.tile([128, 2], F32, tag="encb", name="encb")
        dma(out=encb, in_=enc_b.rearrange("(j p) -> p j", p=128))
        i32s = const.tile([1, 32 * 32], F32, tag="i32s", name="i32s")
        dma(out=i32s, in_=i32f)
        nat = {}

        # ------------- weight prep (emitted lazily, per consuming layer) -------
        # tr[k][p, j, c] = W[c, j*128+p]
        tr = {}

        def prep_tr(k):
            m = const.tile([128, 2, 256], F32, tag=f"nat_{k}", name=f"nat_{k}")
            mr = mats[k].rearrange("(j p) h -> p j h", p=128)
            for j in range(2):  # split: transposes of half j start after half j lands
                dma(out=m[:, j, :], in_=mr[:, j, :])
            nat[k] = m
            t = const.tile([128, 2, 256], F32, tag=f"tr_{k}", name=f"tr_{k}")
            for ji in range(2):  # ji-major: row-tile kk=0 of tr ready first
                for jo in range(2):
                    ps = sp.tile([128, 128], F32, tag="sp", name="sp")
                    nc.tensor.transpose(ps, nat[k][:, ji, jo * 128:(jo + 1) * 128], ids)
                    dve.tensor_copy(t[:, jo, ji * 128:(ji + 1) * 128], ps)
            tr[k] = t

        # WT[i] [2, 256] (f32r) = (ref_wi @ enc_w)^T   (e-matmul lhsT)
        # Wn[i] [p, kt, c]      = ref_wi @ enc_w       (natural)
        # bp[i] [128, 2]        = ref_wi @ enc_b + ref_bi
        WT, Wn, bp, mv, QT, qb = {}, {}, {}, {}, {}, {}

        def prep_ref(i):
            prep_tr(f"ref_w{i}")
            trw = tr[f"ref_w{i}"]
            ps = sp.tile([2, 256], F32, tag="sp", name="sp")
            for kk in range(2):
                mm(ps, lhsT=encs[:, kk, :], rhs=trw[:, kk, :],
                   start=(kk == 0), stop=(kk == 1))
            WT[i] = const.tile([2, 256], F32R, tag=f"WT{i}", name=f"WT{i}")
            dve.tensor_copy(WT[i], ps)

            Wn[i] = const.tile([128, 2, 2], F32, tag=f"Wn{i}", name=f"Wn{i}")
            for j in range(2):
                ps = sp.tile([128, 2], F32, tag="sp", name="sp")
                for kk in range(2):
                    mm(ps, lhsT=trw[:, kk, j * 128:(j + 1) * 128],
                       rhs=encs[:, kk, :], start=(kk == 0), stop=(kk == 1))
                dve.tensor_copy(Wn[i][:, j, :], ps)

            bp[i] = const.tile([128, 2], F32, tag=f"bp{i}", name=f"bp{i}")
            for j in range(2):
                ps = sp.tile([128, 1], F32, tag="sp", name="sp")
                for kk in range(2):
                    mm(ps, lhsT=trw[:, kk, j * 128:(j + 1) * 128],
                       rhs=encb[:, kk:kk + 1], start=(kk == 0), stop=(kk == 1))
                dve.tensor_add(bp[i][:, j:j + 1], ps, vsget(f"ref_b{i}")[:, j:j + 1])

        def prep_mv(i):
            # masked-v weights: mv[i][k, b, m] = v_i[c*128+k] * I[b==m]
            t = const.tile([128, 2, 32, 32], F32R, tag=f"mv{i}", name=f"mv{i}")
            tv = t.rearrange("p c b m -> p (c b m)")
            for c in range(2):
                zt = sp.tile([1, 128], F32, tag="sp", name="sp")
                nc.tensor.transpose(zt, vsget(f"v{i}")[:, c:c + 1], ids)
                vrow = work.tile([1, 128], F32, tag="vrow", name="vrow")
                dve.tensor_copy(vrow, zt)
                for h in range(2):
                    mp = sp.tile([128, 512], F32, tag="sp", name="sp")
                    mm(mp, lhsT=vrow, rhs=i32s[:, h * 512:(h + 1) * 512],
                       start=True, stop=True)
                    dve.tensor_copy(tv[:, (2 * c + h) * 512:(2 * c + h + 1) * 512], mp)
            mv[i] = t

        def prep_q(i):
            # QT[i] [2,256] = (q_wi @ W'_{i-1})^T; qb[i] = q_wi@b'_{i-1}+q_bi
            prep_tr(f"q_w{i}")
            trq = tr[f"q_w{i}"]
            ps = sp.tile([2, 256], F32, tag="sp", name="sp")
            for kk in range(2):
                mm(ps, lhsT=Wn[i - 1][:, kk, :], rhs=trq[:, kk, :],
                   start=(kk == 0), stop=(kk == 1))
            QT[i] = const.tile([2, 256], F32, tag=f"QT{i}", name=f"QT{i}")
            dve.tensor_copy(QT[i], ps)

            qb[i] = const.tile([128, 2], F32, tag=f"qb{i}", name=f"qb{i}")
            for j in range(2):
                ps = sp.tile([128, 1], F32, tag="sp", name="sp")
                for kk in range(2):
                    mm(ps, lhsT=trq[:, kk, j * 128:(j + 1) * 128],
                       rhs=bp[i - 1][:, kk:kk + 1], start=(kk == 0), stop=(kk == 1))
                dve.tensor_add(qb[i][:, j:j + 1], ps, vsget(f"q_b{i}")[:, j:j + 1])

        x0s = x1s = None  # created during layer-1 group 0

        # minimal prep before layer-1 compute can start
        prep_ref(1)
        qeff1 = const.tile([128, 2], F32, tag="qeff1", name="qeff1")
        dve.tensor_add(qeff1, bp[1], vsget("q_b1"))
        prep_mv(1)

        # ---------------- main layers ----------------
        Zs = None
        for li in (1, 2, 3):
            if li == 1:
                qeff_tile, qoff = qeff1, lambda c, b: c
            else:
                qeff = work.tile([128, 2 * BC], F32, tag="qeff", name="qeff")
                for c in range(2):
                    qp = sp.tile([128, BC], F32, tag="sp", name="sp")
                    mm(qp, lhsT=QT[li][:, c * 128:(c + 1) * 128], rhs=Zs,
                       start=True, stop=True)
                    dve.tensor_scalar_add(qeff[:, c * BC:(c + 1) * BC], qp,
                                          qb[li][:, c:c + 1])
                qeff_tile, qoff = qeff, lambda c, b: c * BC + b
            # ACT touch: fold the DVE tick for qeff into ACT's clock so the
            # first tanh needs only its PE wait.

            U = up.tile([BC, N], F32, tag="U", name="U")
            mvl = mv[li]
            pend_u = []
            ustate = {"first": True}

            def emit_u(t, c, b, _mvl=mvl, _U=U, _ustate=None):
                st = ustate if _ustate is None else _ustate
                first = st["first"]
                st["first"] = False
                last = (b == BC - 1 and c == 1)
                for n0, n1 in zip(NCH[:-1], NCH[1:]):
                    mm(_U[:, n0:n1], lhsT=_mvl[:, c, b, :],
                       rhs=t[:, n0:n1], start=first, stop=last)
            import os as _os
            _ng = int(_os.environ.get("NGROUPS", str(BC // GB)))
            for g in range(_ng):
                cg = scr.tile([2, GB, N], F32R, tag="cg", name="cg")
                dma(out=cg,
                    in_=x[2 * GB * g:2 * GB * (g + 1), :].rearrange(
                        "(g c) n -> c g n", c=2).bitcast(F32R))
                for gi in range(GB):
                    b = g * GB + gi
                    for c in range(2):
                        pe = ep.tile([128, N], F32, tag="e", name="e")
                        lw = WT[li][:, c * 128:(c + 1) * 128]
                        for n0, n1 in zip(NCH[:-1], NCH[1:]):
                            mm(pe[:, n0:n1], lhsT=lw, rhs=cg[:, gi, n0:n1],
                               start=True, stop=True)
                        t = tp.tile([128, N], F32R, tag="t", name="t")
                        act(t, pe, AF.Tanh,
                            bias=qeff_tile[:, qoff(c, b):qoff(c, b) + 1])
                        # defer this (b, c)'s u-matmuls two tanh ops behind:
                        # gives the tanh -> u-matmul semaphore edge slack so
                        # PE never polls a not-yet-posted semaphore
                        pend_u.append((t, c, b))
                        if len(pend_u) > 2:
                            emit_u(*pend_u.pop(0))
                if g == 0 and li == 1:
                    # softmax inputs, needed from ~the end of layer 1 on
                    x0s = const.tile([BC, N], F32, tag="x0s", name="x0s")
                    dma(out=x0s, in_=x0)
                    x1s = const.tile([BC, N], F32, tag="x1s", name="x1s")
                    dma(out=x1s, in_=x1)
                if g == 0 and li < 3:
                    # emit the next layer's weight prep here so it lands
                    # mid-layer in each engine's static order, filling PE
                    # slack under the ACT-bound steady state
                    prep_ref(li + 1)
                    prep_mv(li + 1)
                    prep_q(li + 1)

            while pend_u:
                emit_u(*pend_u.pop(0))

            # batched softmax over N + z = (p*X).sum / sum(p).
            # u = v . tanh(...) is bounded (|u| < ~4), so exp needs no
            # max-subtraction; skipping it shortens the layer boundary.
            P = ppool.tile([BC, N], F32, tag="P", name="P")
            ssum = work.tile([BC, 1], F32, tag="ssum", name="ssum")
            act(P, U, AF.Exp, accum_out=ssum)
            rinv = work.tile([BC, 1], F32, tag="rinv", name="rinv")
            dve.reciprocal(rinv, ssum)
            s0 = work.tile([BC, 1], F32, tag="s0", name="s0")
            s1 = work.tile([BC, 1], F32, tag="s1", name="s1")
            pxs = scr.tile([BC, N], F32, tag="pxs", name="pxs")
            dve.scalar_tensor_tensor(out=pxs, in0=P, scalar=1.0, in1=x0s,
                                     op0=ALU.mult, op1=ALU.mult, accum_out=s0)
            pxs2 = scr.tile([BC, N], F32, tag="pxs2", name="pxs2")
            dve.scalar_tensor_tensor(out=pxs2, in0=P, scalar=1.0, in1=x1s,
                                     op0=ALU.mult, op1=ALU.mult, accum_out=s1)
            spair = work.tile([BC, 2], F32, tag="spair", name="spair")
            dve.tensor_mul(spair[:, 0:1], s0, rinv)
            dve.tensor_mul(spair[:, 1:2], s1, rinv)
            zp = sp.tile([2, BC], F32, tag="sp", name="sp")
            nc.tensor.transpose(zp, spair, ids[0:BC, 0:BC])
            Zs = work.tile([2, BC], F32, tag="Z", name="Z")
            dve.tensor_copy(Zs, zp)

        # head weights: FT [2,256] = (fc1_w @ W'_3)^T, fb = fc1_w @ b'_3 + fc1_b
        fc2s = const.tile([128, 2], F32, tag="fc2s", name="fc2s")
        dma(out=fc2s, in_=fc2_w.rearrange("a (j p) -> p (a j)", p=128))
        fc2bs = const.tile([1, 1], F32, tag="fc2bs", name="fc2bs")
        dma(out=fc2bs, in_=fc2_b.unsqueeze(1))
        prep_tr("fc1_w")
        trf = tr["fc1_w"]
        ps = sp.tile([2, 256], F32, tag="sp", name="sp")
        for kk in range(2):
            mm(ps, lhsT=Wn[3][:, kk, :], rhs=trf[:, kk, :],
               start=(kk == 0), stop=(kk == 1))
        FT = const.tile([2, 256], F32, tag="FT", name="FT")
        dve.tensor_copy(FT, ps)
        fb = const.tile([128, 2], F32, tag="fb", name="fb")
        for j in range(2):
            ps = sp.tile([128, 1], F32, tag="sp", name="sp")
            for kk in range(2):
                mm(ps, lhsT=trf[:, kk, j * 128:(j + 1) * 128],
                   rhs=bp[3][:, kk:kk + 1], start=(kk == 0), stop=(kk == 1))
            dve.tensor_add(fb[:, j:j + 1], ps, vsget("fc1_b")[:, j:j + 1])

        # ---------------- head ----------------
        Rt = []
        for c in range(2):
            ap_ = sp.tile([128, BC], F32, tag="sp", name="sp")
            mm(ap_, lhsT=FT[:, c * 128:(c + 1) * 128], rhs=Zs, start=True, stop=True)
            r = work.tile([128, BC], F32, tag=f"R{c}", name=f"R{c}")
            act(r, ap_, AF.Relu, bias=fb[:, c:c + 1])
            Rt.append(r)
        op = sp.tile([1, BC], F32, tag="sp", name="sp")
        for c in range(2):
            mm(op, lhsT=fc2s[:, c:c + 1], rhs=Rt[c], start=(c == 0), stop=(c == 1))
        osb = work.tile([1, BC], F32, tag="osb", name="osb")
        dve.tensor_scalar_add(osb, op, fc2bs[0:1, 0:1])
        dma(out=out.unsqueeze(0), in_=osb)

    _split_multi_waits(nc)
    return nc


_NC = None


def _get_nc():
    global _NC
    if _NC is None:
        _NC = build_nc()
    return _NC


def make_in_maps(inputs):
    """Shard the full inputs into per-core in_maps for run_bass_kernel_spmd."""
    ins = {k: np.ascontiguousarray(np.asarray(v, dtype=np.float32))
           for k, v in inputs.items()}
    static = ins["static"]
    assert static.shape == (B, N, 2)
    ident = np.eye(128, dtype=np.float32)
    i32 = np.eye(32, dtype=np.float32).reshape(1, 1024)
    shared = {k: ins[k] for k in ins if k != "static" and k != "q_w1"}
    in_maps = []
    for c in range(NCORES):
        sh = np.ascontiguousarray(static[c * BC:(c + 1) * BC])
        m = dict(shared)
        xr = sh.reshape(2 * BC, N)  # raw reshape, matches the reference
        m["x"] = xr
        m["x0"] = np.ascontiguousarray(xr[0::2])
        m["x1"] = np.ascontiguousarray(xr[1::2])
        m["ident"] = ident
        m["i32f"] = i32
        in_maps.append(m)
    return in_maps


def kernel(**inputs) -> np.ndarray:
    nc = _get_nc()
    in_maps = make_in_maps(inputs)
    res = run_bass_kernel_spmd(nc, in_maps, list(range(NCORES)))
    outs = [np.asarray(res.results[c]["out"], dtype=np.float32)
            for c in range(NCORES)]
    return np.concatenate(outs, axis=0).reshape(B, 1)



# revision 2
# speedup vs baseline: 1.0008x; 1.0008x over previous
"""Trainium2 Bass kernel for the Critic model (attention-pointer critic), v3.

Math (per batch b, coords = raw-reshape(static[b]) as [2, N]):
    sh  = enc_w @ coords + enc_b                       [H, N]
    for layer i in 1..3:
        e_i  = ref_wi @ sh + ref_bi                    [H, N]
        q_i  = q_wi @ hy + q_bi                        [H]
        u_i  = v_i . tanh(e_i + q_i)                   [N]
        p_i  = softmax(u_i)
        hy   = e_i @ p_i                               [H]
    out = fc2 @ relu(fc1 @ hy + fc1_b) + fc2_b         [1]

All linear folding is done HOST-side in make_in_maps (numpy):
    W_i   = ref_wi @ enc_w                [H, 2]
    b_i   = ref_wi @ enc_b + ref_bi      [H]
    tanh argument for (layer i, batch b) = W_i @ x_b + (b_i + q_i(z)) 1^T
      where z = coords @ softmax(u) (2-vector), q_1 = q_b1 (hy0 = 0),
      q_i(z) = (q_wi @ W_{i-1}) z + (q_wi b_{i-1} + q_bi)  for i >= 2.
    The additive term rides as a THIRD contraction row: device matmuls use
    lhsT = [W_i[:,0]; W_i[:,1]; qeff]  (K=3) against rhs = [x0; x1; 1].

Device work per (batch, half-H "pair", n-half "slot"; 128 slots/layer/core):
    e+q  = lhsT.T @ [x0;x1;1]    K=3 matmul, row-tiled 3-concurrent (PE)
    t    = tanh(e+q)             ONE ACT instr per 3 slots ([128,1536], 3 banks)
    u   += masked-v matmul       K=128 into U [32,1024] psum (2 banks, n-halved)
then per layer: exp+softmax sums (ACT/DVE), z, next layer's qeff fold:
    qeff = QX^T @ [z;1] (PE) -> transpose -> SBUF->SBUF DMA into the lhsT
    tables' third rows (partitions 2/34/66; replicated per PE row-group).

PSUM budget: e-tiles 2 bufs x 3 banks + U 2 banks = 8; boundary/head
scratch borrows e-pool slots (only alive between layers).

Walrus quirk handled by _split_multi_waits (unchanged from the previous
kernel): at most one sync wait per instruction struct; extra waits hoist to
standalone InstEventSemaphore; wide semaphore range-clears are chunked;
custom DVE ops carry no embedded sync.

Sharding: pure data-parallel, 32 batches/core x 8 cores, weights replicated.
"""

import sys

if "/opt/trn_rl_repo" not in sys.path:
    sys.path.insert(0, "/opt/trn_rl_repo")

from contextlib import ExitStack

import numpy as np

import concourse.bass as bass
import concourse.tile as tile
from concourse import mybir
from concourse.bass import _add_dep_helper
from concourse.bass_utils import run_bass_kernel_spmd

B, N, H = 256, 1000, 256
NCORES = 8
BC = B // NCORES   # batches per core
GB = 8             # batches per coords tile
NG = BC // GB      # coords tiles per pass
NSLOT = 2 * BC * 2  # (c, b) pairs x n-halves per layer
GRP = 3            # slots per tanh instruction / psum e-tile

F32 = mybir.dt.float32
F32R = mybir.dt.float32r
AF = mybir.ActivationFunctionType
ALU = mybir.AluOpType

NP = 1024  # padded N: host ships x3 zero-padded so every slot is 512 wide
HALF = ((0, 512), (512, 512))  # (start col, width) of the two n-halves


def _split_multi_waits(nc):
    """Walrus in this container accepts at most one sync wait per
    instruction struct. Hoist extra waits onto standalone InstEventSemaphore
    instructions inserted just before the owner (engines are in-order, so the
    semantics are identical)."""
    import os
    split_max = int(os.environ.get("SPLIT_MAX", "999999"))
    nsofar = [0]

    def mk_ev(inst, w):
        ev = mybir.InstEventSemaphore(name=nc.get_next_instruction_name())
        ev.engine = inst.engine
        ev.sync_info = mybir.SyncInfo(on_wait=[w], on_update=[])
        ev.debug = mybir.OpDebugInfo(
            op_name=f"splitwait:{inst.name}:{w.ant_name}",
            filename="kernel.py", lineno=1)
        nc.register_instruction(ev)
        return ev

    f = nc.m.functions[0]
    blocks = list(f.blocks)

    # EVENT_SEMAPHORE_RANGE_CLEAR supports at most 8 semaphores per
    # instruction on this walrus; chunk wider ranges.
    for blk in blocks:
        old_insts = blk.instructions
        rewritten = []
        changed = False
        for inst in old_insts:
            if (type(inst).__name__ == "InstISA"
                    and inst.op_name == "EVENT_SEMAPHORE_RANGE_CLEAR"):
                d = dict(inst.ant_dict)
                first, last = d["range_first"], d["range_last"]
                if last - first + 1 > 8:
                    changed = True
                    lo = first
                    while lo <= last:
                        hi = min(lo + 7, last)
                        nb = list(inst.instr)
                        nb[13], nb[14] = lo, hi
                        d2 = dict(d)
                        d2["range_first"], d2["range_last"] = lo, hi
                        ni = mybir.InstISA(
                            name=nc.get_next_instruction_name(),
                            isa_opcode=inst.isa_opcode,
                            engine=inst.engine,
                            instr=nb,
                            op_name=inst.op_name,
                            ins=[], outs=[],
                            ant_dict=d2,
                            verify=inst.verify,
                            ant_isa_is_sequencer_only=inst.ant_isa_is_sequencer_only,
                        )
                        if inst.sync_info is not None and lo == first:
                            ni.sync_info = inst.sync_info
                        nc.register_instruction(ni)
                        rewritten.append(ni)
                        lo = hi + 1
                    continue
            rewritten.append(inst)
        if changed:
            blk.instructions = rewritten

    for bi, blk in enumerate(blocks):
        old = blk.instructions
        if not any(i.sync_info is not None and len(i.sync_info.on_wait) > 1
                   for i in old):
            continue
        new = []
        hoist_prev = []  # evsems that must run before this block is entered
        for idx, inst in enumerate(old):
            si = inst.sync_info
            is_custom = type(inst).__name__ in ("InstReciprocal",)
            if si is not None and is_custom and (si.on_wait or si.on_update):
                # custom-DVE ops lower to fixed-length ISA payloads that
                # cannot carry embedded sync: hoist waits before, updates
                # after (engine is in-order, semantics unchanged).
                for w in si.on_wait:
                    new.append(mk_ev(inst, w))
                posts = list(si.on_update)
                inst.sync_info = mybir.SyncInfo(on_wait=[], on_update=[])
                new.append(inst)
                for u in posts:
                    ev = mybir.InstEventSemaphore(
                        name=nc.get_next_instruction_name())
                    ev.engine = inst.engine
                    ev.sync_info = mybir.SyncInfo(on_wait=[], on_update=[u])
                    ev.debug = mybir.OpDebugInfo(
                        op_name=f"splitupd:{inst.name}",
                        filename="kernel.py", lineno=1)
                    nc.register_instruction(ev)
                    new.append(ev)
                continue
            if si is not None and len(si.on_wait) > 1 and nsofar[0] < split_max:
                nsofar[0] += 1
                waits = list(si.on_wait)
                evs = [mk_ev(inst, w) for w in waits[:-1]]
                if idx == 0 and bi > 0 and type(inst).__name__ == "InstDrain":
                    # barrier-teardown block: walrus rejects extra
                    # instructions before the first drain, so run the waits
                    # at the tail of the previous block instead.
                    hoist_prev.extend(evs)
                else:
                    new.extend(evs)
                inst.sync_info = mybir.SyncInfo(on_wait=[waits[-1]],
                                                on_update=list(si.on_update))
            new.append(inst)
        blk.instructions = new
        if hoist_prev:
            prev = blocks[bi - 1]
            pinsts = prev.instructions
            cut = len(pinsts)
            while cut > 0 and "Branch" in type(pinsts[cut - 1]).__name__:
                cut -= 1
            prev.instructions = pinsts[:cut] + hoist_prev + pinsts[cut:]


def build_nc():
    nc = bass.Bass(trn_type="TRN2", target_bir_lowering=False)

    def din(name, shape):
        return nc.dram_tensor(name, shape, F32, kind="ExternalInput").ap()

    x9 = din("x9", [9 * BC, NP])  # per tile k: 3 row-group replicas of [x0;x1;1] x GB batches
    xz = din("xz", [BC, 2 * N])   # [x0 | x1], b-partition layout
    ident = din("ident", [128, 128])
    lw = {i: din(f"lw{i}", [3, 2 * BC * 128]) for i in (1, 2, 3)}
    qfx = din("qfx", [3, 3 * H + 1])  # [qx2 | qx3 | fx | fc2b-col]
    mv_in = {i: din(f"mv{i}", [128, 2 * BC * BC]) for i in (1, 2, 3)}
    fc2s_in = din("fc2sw", [128, 2])
    out = nc.dram_tensor("out", [BC], F32, kind="ExternalOutput").ap()

    with ExitStack() as ctx:
        tc = ctx.enter_context(tile.TileContext(nc))
        const = ctx.enter_context(tc.tile_pool(name="const", bufs=1))
        lwp = ctx.enter_context(tc.tile_pool(name="lwp", bufs=2))
        cgp = ctx.enter_context(tc.tile_pool(name="cgp", bufs=2))
        tp = ctx.enter_context(tc.tile_pool(name="tp", bufs=2))
        mvp = ctx.enter_context(tc.tile_pool(name="mvp", bufs=2))
        wk = ctx.enter_context(tc.tile_pool(name="wk", bufs=2))
        ep = ctx.enter_context(tc.tile_pool(name="ep", bufs=2, space="PSUM"))
        up = ctx.enter_context(tc.tile_pool(name="up", bufs=1, space="PSUM"))

        mm = nc.tensor.matmul
        act = nc.scalar.activation
        dve = nc.vector
        dma = nc.sync.dma_start      # HWDGE via SP: coords + boundary writes
        wdma = nc.gpsimd.dma_start   # SWDGE via Pool: weights (off SP's FIFO)

        # ---------------- constants / weights (layer-1 critical path first) --
        lwt = {}

        def load_lwt(i, eng=None):
            # write partitions {0-2, 32-34, 64-66}: the three PE row-group
            # replicas of the per-pair lhsT table
            t = lwp.tile([96, 2 * BC * 128], F32R, tag="lwt", name=f"lwt{i}")
            for g in range(3):
                (eng or wdma)(out=t[32 * g:32 * g + 3, :],
                              in_=lw[i].bitcast(F32R))
            lwt[i] = t

        mvs = {}

        def load_mv(i):
            t = mvp.tile([128, 2 * BC * BC], F32R, tag="mv", name=f"mv{i}")
            for c in range(2):
                wdma(out=t[:, c * BC * BC:(c + 1) * BC * BC],
                    in_=mv_in[i][:, c * BC * BC:(c + 1) * BC * BC].bitcast(F32R))
            mvs[i] = t

        load_lwt(1)

        deferred = []  # emitted after the first coords tile's DMA

        def defer_consts():
            ids = const.tile([128, 128], F32, tag="ids", name="ids")
            dma(out=ids, in_=ident)
            # touch tanh ASAP so the ACT table-set load (~2.7us on HW)
            # overlaps the startup DMAs instead of the first real tanh
            warm = wk.tile([1, 1], F32, tag="warm", name="warm")
            act(warm, ids[0:1, 0:1], AF.Tanh)
            qf = const.tile([3, 3 * H + 1], F32, tag="qf", name="qf")
            dma(out=qf, in_=qfx)
            fc2s = const.tile([128, 2], F32, tag="fc2s", name="fc2s")
            dma(out=fc2s, in_=fc2s_in)
            xzs = const.tile([BC, 2 * N], F32, tag="xzs", name="xzs")
            dma(out=xzs, in_=xz)
            zs1 = const.tile([3, BC], F32, tag="zs1", name="zs1")
            dma(out=zs1[2:3, :], in_=x9[2 * GB:2 * GB + 1, 0:BC])  # ones row
            deferred.extend([ids, qf, fc2s, xzs, zs1])

        qxs_sl = {2: (0, H), 3: (H, 2 * H)}

        cg_tiles = {}

        def get_cg(k):
            if k >= 3 * NG:
                return None
            if k not in cg_tiles:
                t = cgp.tile([96, GB, NP], F32R, tag="cg", name="cg")
                kk = k % NG
                for a in range(3):
                    s = x9[9 * GB * kk + 3 * GB * a:
                           9 * GB * kk + 3 * GB * (a + 1), :]
                    dma(out=t[32 * a:32 * a + 3, :, :],
                        in_=s.rearrange("(c g) n -> c g n", c=3).bitcast(F32R))
                cg_tiles[k] = t
            return cg_tiles[k]

        get_cg(0)
        load_mv(1)
        get_cg(1)
        defer_consts()
        ids, qf, fc2s, xzs, zs1 = deferred

        # ---------------- layers ----------------
        for li in (1, 2, 3):
            U = up.tile([32, 1024], F32, tag="U", name="U")
            P = wk.tile([BC, N], F32, tag="P", name="P")
            # stats cols: 0 es0, 1 es1, 2 s00, 3 s01, 4 s10, 5 s11,
            #             6 ssum, 7 s0, 8 s1, 9 rinv, 10 z0, 11 z1
            st = wk.tile([BC, 12], F32, tag="st", name="st")
            mvl = mvs[li]
            lwl = lwt[li]


            ucnt = [0, 0]      # u-matmuls emitted per n-half
            pend = []          # [(t_tile, group), ...] deferred u-matmul work
            expq = []          # deferred half-0 softmax emission
            group = []         # (j, cb, gi, h) of current e-tile
            pe_cur = [None]

            def emit_us(t, grp):
                for (j, cb, gi_, h) in grp:
                    hs, hw = HALF[h]
                    cnt = ucnt[h]
                    ucnt[h] += 1
                    mm(U[:, hs:hs + hw], lhsT=mvl[:, cb * BC:(cb + 1) * BC],
                       rhs=t[:, j * 512:j * 512 + hw],
                       start=(cnt == 0), stop=(cnt == 2 * BC - 1))
                    if h == 0 and cnt == 2 * BC - 1:
                        expq.append(2)  # emit half-0 softmax 2 flushes later

            def emit_exp_half(h):
                hs, hw = HALF[h]
                hw = min(hw, N - hs)
                act(P[:, hs:hs + hw], U[:, hs:hs + hw], AF.Exp,
                    accum_out=st[:, 3 * h:3 * h + 1])
                junk = wk.tile([BC, 512], F32, tag="pxs", name="pxs")
                dve.scalar_tensor_tensor(
                    out=junk[:, 0:hw], in0=P[:, hs:hs + hw], scalar=1.0,
                    in1=xzs[:, hs:hs + hw], op0=ALU.mult, op1=ALU.mult,
                    accum_out=st[:, 3 * h + 1:3 * h + 2])
                junk2 = wk.tile([BC, 512], F32, tag="pxs2", name="pxs2")
                dve.scalar_tensor_tensor(
                    out=junk2[:, 0:hw], in0=P[:, hs:hs + hw], scalar=1.0,
                    in1=xzs[:, N + hs:N + hs + hw], op0=ALU.mult, op1=ALU.mult,
                    accum_out=st[:, 3 * h + 2:3 * h + 3])

            def flush():
                if not group:
                    return
                g = len(group)
                t = tp.tile([128, GRP * 512], F32R, tag="t", name="t")
                act(t[:, :g * 512], pe_cur[0][:, :g * 512], AF.Tanh)
                pend.append((t, list(group)))
                group.clear()
                pe_cur[0] = None
                if len(pend) > 1:
                    tt, grp = pend.pop(0)
                    emit_us(tt, grp)
                for i in range(len(expq)):
                    expq[i] -= 1
                if expq and expq[0] <= 0:
                    expq.pop(0)
                    emit_exp_half(0)

            for k in range(NG):
                cg = get_cg((li - 1) * NG + k)
                get_cg((li - 1) * NG + k + 1)  # prefetch (incl. next layer's)
                if li == 1 and k == 1:
                    # prefetch later layers' weights mid-layer-1
                    load_lwt(2)
                    load_mv(2)
                if li == 2 and k == 1:
                    load_lwt(3)
                    load_mv(3)
                for h in range(2):
                    hs, hw = HALF[h]
                    for gi in range(GB):
                        b = k * GB + gi
                        for c in range(2):
                            cb = c * BC + b
                            j = len(group)
                            if j == 0:
                                pe_cur[0] = ep.tile([128, GRP * 512], F32,
                                                    tag="pe", name="pe")
                            mm(pe_cur[0][:, j * 512:j * 512 + hw],
                               lhsT=lwl[32 * j:32 * j + 3,
                                        cb * 128:(cb + 1) * 128],
                               rhs=cg[32 * j:32 * j + 3, gi, hs:hs + hw],
                               start=True, stop=True)
                            group.append((j, cb, gi, h))
                            if len(group) == GRP:
                                flush()
            flush()
            while pend:
                tt, grp = pend.pop(0)
                emit_us(tt, grp)
            while expq:
                expq.pop(0)
                emit_exp_half(0)

            # ---- layer end: half-1 softmax, z, next-layer qeff fold ----
            emit_exp_half(1)
            # ssum first, reciprocal EARLY; the s0/s1 sums are emitted after
            # it so the spair muls' same-engine wait count covers the custom
            # op's out-of-band completion (same shape the prior kernel used)
            dve.tensor_tensor(out=st[:, 6:7], in0=st[:, 0:1], in1=st[:, 3:4],
                              op=ALU.add)
            rinv = wk.tile([BC, 1], F32, tag="rinv", name="rinv")
            dve.reciprocal(rinv, st[:, 6:7])
            dve.tensor_tensor(out=st[:, 7:9], in0=st[:, 1:3], in1=st[:, 4:6],
                              op=ALU.add)
            spair = wk.tile([BC, 2], F32, tag="spair", name="spair")
            dve.tensor_scalar(out=spair, in0=st[:, 7:9], scalar1=rinv,
                              scalar2=None, op0=ALU.mult)
            zp = ep.tile([2, BC], F32, tag="pe", name="zp")
            nc.tensor.transpose(zp, spair, ids[0:BC, 0:BC])
            dve.tensor_copy(zs1[0:2, :], zp)

            if li < 3:
                # qeffT[cb, h] directly: per c-half, out partitions c*32..+32
                q0, q1 = qxs_sl[li + 1]
                qtp = ep.tile([2 * BC, 128], F32, tag="pe", name="qtp")
                for c in range(2):
                    mm(qtp[c * BC:(c + 1) * BC, :], lhsT=zs1,
                       rhs=qf[:, q0 + c * 128:q0 + (c + 1) * 128],
                       start=True, stop=True)
                qtb = wk.tile([2 * BC, 128], F32, tag="qtb", name="qtb")
                dve.tensor_copy(qtb, qtp)
                for g in range(3):
                    dma(out=lwt[li + 1][32 * g + 2:32 * g + 3, :],
                        in_=qtb.bitcast(F32R))

        # ---------------- head ----------------
        rt = []
        for c in range(2):
            ap_ = ep.tile([128, BC], F32, tag="pe", name="hp")
            mm(ap_, lhsT=qf[:, 2 * H + c * 128:2 * H + (c + 1) * 128],
               rhs=zs1, start=True, stop=True)
            r = wk.tile([128, BC], F32, tag=f"R{c}", name=f"R{c}")
            act(r, ap_, AF.Relu)
            rt.append(r)
        op = ep.tile([1, BC], F32, tag="pe", name="op")
        for c in range(2):
            mm(op, lhsT=fc2s[:, c:c + 1], rhs=rt[c],
               start=(c == 0), stop=(c == 1))
        osb = wk.tile([1, BC], F32, tag="osb", name="osb")
        dve.tensor_scalar_add(osb, op, qf[0:1, 3 * H:3 * H + 1])
        dma(out=out.unsqueeze(0), in_=osb)

    _split_multi_waits(nc)
    return nc


_NC = None


def _get_nc():
    global _NC
    if _NC is None:
        _NC = build_nc()
    return _NC


def make_in_maps(inputs):
    """Host-side fold of all weight algebra + sharding into per-core maps."""
    f = np.float32
    ins = {k: np.ascontiguousarray(np.asarray(v, dtype=f))
           for k, v in inputs.items()}
    static = ins["static"]
    assert static.shape == (B, N, 2)
    enc_w, enc_b = ins["enc_w"], ins["enc_b"]

    W, bb = {}, {}
    for i in (1, 2, 3):
        W[i] = ins[f"ref_w{i}"] @ enc_w                      # [H, 2]
        bb[i] = ins[f"ref_w{i}"] @ enc_b + ins[f"ref_b{i}"]  # [H]

    def build_lw(i, qrow):
        # lw[k, cb*128 + h]: rows 0,1 = W_i[c*128+h, k]; row 2 = qrow[c*128+h]
        arr = np.zeros((3, 2, BC, 128), dtype=f)
        for c in (0, 1):
            blk = W[i][c * 128:(c + 1) * 128, :]             # [128, 2]
            arr[0, c, :, :] = blk[:, 0][None, :]
            arr[1, c, :, :] = blk[:, 1][None, :]
            if qrow is not None:
                arr[2, c, :, :] = qrow[c * 128:(c + 1) * 128][None, :]
        return np.ascontiguousarray(arr.reshape(3, 2 * BC * 128))

    qrow1 = bb[1] + ins["q_b1"]
    lw = {1: build_lw(1, qrow1), 2: build_lw(2, None), 3: build_lw(3, None)}

    qfx = np.zeros((3, 3 * H + 1), dtype=f)
    for i in (2, 3):
        qw = ins[f"q_w{i}"]
        m = np.zeros((3, H), dtype=f)
        m[0:2, :] = (qw @ W[i - 1]).T
        m[2, :] = bb[i] + qw @ bb[i - 1] + ins[f"q_b{i}"]
        qfx[:, (i - 2) * H:(i - 1) * H] = m
    qfx[0:2, 2 * H:3 * H] = (ins["fc1_w"] @ W[3]).T
    qfx[2, 2 * H:3 * H] = ins["fc1_w"] @ bb[3] + ins["fc1_b"]
    qfx[0, 3 * H] = ins["fc2_b"][0]

    mv = {}
    for i in (1, 2, 3):
        v = ins[f"v{i}"]
        m = np.zeros((128, 2, BC, BC), dtype=f)
        for c in (0, 1):
            m[:, c, np.arange(BC), np.arange(BC)] = v[c * 128:(c + 1) * 128, None]
        mv[i] = np.ascontiguousarray(m.reshape(128, 2 * BC * BC))

    fc2s = np.ascontiguousarray(
        ins["fc2_w"].reshape(2, 128).T)               # fc2s[p, j] = w[0, j*128+p]
    ident = np.eye(128, dtype=f)

    shared = {"ident": ident, "qfx": qfx, "fc2sw": fc2s}
    for i in (1, 2, 3):
        shared[f"lw{i}"] = lw[i]
        shared[f"mv{i}"] = mv[i]

    in_maps = []
    for core in range(NCORES):
        sh = static[core * BC:(core + 1) * BC]
        xr = sh.reshape(2 * BC, N)        # raw reshape, matches the reference
        blk = np.zeros((NG, 3, GB, NP), dtype=f)
        blk[:, 0, :, :N] = xr[0::2].reshape(NG, GB, N)
        blk[:, 1, :, :N] = xr[1::2].reshape(NG, GB, N)
        blk[:, 2, :, :N] = 1.0
        x9c = np.ascontiguousarray(
            np.broadcast_to(blk[:, None], (NG, 3, 3, GB, NP))
        ).reshape(9 * BC, NP)
        m = dict(shared)
        m["x9"] = x9c
        xzc = np.empty((BC, 2 * N), dtype=f)
        xzc[:, :N] = xr[0::2]
        xzc[:, N:] = xr[1::2]
        m["xz"] = xzc
        in_maps.append(m)
    return in_maps


def kernel(**inputs) -> np.ndarray:
    nc = _get_nc()
    in_maps = make_in_maps(inputs)
    res = run_bass_kernel_spmd(nc, in_maps, list(range(NCORES)))
    outs = [np.asarray(res.results[c]["out"], dtype=np.float32)
            for c in range(NCORES)]
    return np.concatenate(outs, axis=0).reshape(B, 1)


# revision 18
# speedup vs baseline: 1.0547x; 1.0539x over previous
"""Trainium2 Bass kernel for the Critic model (attention-pointer critic), v3.

Math (per batch b, coords = raw-reshape(static[b]) as [2, N]):
    sh  = enc_w @ coords + enc_b                       [H, N]
    for layer i in 1..3:
        e_i  = ref_wi @ sh + ref_bi                    [H, N]
        q_i  = q_wi @ hy + q_bi                        [H]
        u_i  = v_i . tanh(e_i + q_i)                   [N]
        p_i  = softmax(u_i)
        hy   = e_i @ p_i                               [H]
    out = fc2 @ relu(fc1 @ hy + fc1_b) + fc2_b         [1]

All linear folding is done HOST-side in make_in_maps (numpy):
    W_i   = ref_wi @ enc_w                [H, 2]
    b_i   = ref_wi @ enc_b + ref_bi      [H]
    tanh argument for (layer i, batch b) = W_i @ x_b + (b_i + q_i(z)) 1^T
      where z = coords @ softmax(u) (2-vector), q_1 = q_b1 (hy0 = 0),
      q_i(z) = (q_wi @ W_{i-1}) z + (q_wi b_{i-1} + q_bi)  for i >= 2.
    The additive term rides as a THIRD contraction row: device matmuls use
    lhsT = [W_i[:,0]; W_i[:,1]; qeff]  (K=3) against rhs = [x0; x1; 1].

Device work per (batch, half-H "pair", n-half "slot"; 128 slots/layer/core):
    e+q  = lhsT.T @ [x0;x1;1]    K=3 matmul, row-tiled 3-concurrent (PE)
    t    = tanh(e+q)             ONE ACT instr per 3 slots ([128,1536], 3 banks)
    u   += masked-v matmul       K=128 into U [32,1024] psum (2 banks, n-halved)
then per layer: exp+softmax sums (ACT/DVE), z, next layer's qeff fold:
    qeff = QX^T @ [z;1] (PE) -> transpose -> SBUF->SBUF DMA into the lhsT
    tables' third rows (partitions 2/34/66; replicated per PE row-group).

PSUM budget: e-tiles 2 bufs x 3 banks + U 2 banks = 8; boundary/head
scratch borrows e-pool slots (only alive between layers).

Walrus quirk handled by _split_multi_waits (unchanged from the previous
kernel): at most one sync wait per instruction struct; extra waits hoist to
standalone InstEventSemaphore; wide semaphore range-clears are chunked;
custom DVE ops carry no embedded sync.

Sharding: pure data-parallel, 32 batches/core x 8 cores, weights replicated.
"""

import sys

if "/opt/trn_rl_repo" not in sys.path:
    sys.path.insert(0, "/opt/trn_rl_repo")

from contextlib import ExitStack

import numpy as np

import concourse.bass as bass
import concourse.tile as tile
from concourse import mybir
from concourse.bass import _add_dep_helper
from concourse.bass_utils import run_bass_kernel_spmd

B, N, H = 256, 1000, 256
NCORES = 8
BC = B // NCORES   # batches per core
GB = 8             # batches per coords tile
NG = BC // GB      # coords tiles per pass
NSLOT = 2 * BC * 2  # (c, b) pairs x n-halves per layer
GRP = 3            # slots per tanh instruction / psum e-tile

F32 = mybir.dt.float32
F32R = mybir.dt.float32r
AF = mybir.ActivationFunctionType
ALU = mybir.AluOpType

NP = 1024  # padded N: host ships x3 zero-padded so every slot is 512 wide
HALF = ((0, 512), (512, 512))  # (start col, width) of the two n-halves


def _split_multi_waits(nc):
    """Walrus in this container accepts at most one sync wait per
    instruction struct. Hoist extra waits onto standalone InstEventSemaphore
    instructions inserted just before the owner (engines are in-order, so the
    semantics are identical)."""
    import os
    split_max = int(os.environ.get("SPLIT_MAX", "999999"))
    nsofar = [0]

    def mk_ev(inst, w):
        ev = mybir.InstEventSemaphore(name=nc.get_next_instruction_name())
        ev.engine = inst.engine
        ev.sync_info = mybir.SyncInfo(on_wait=[w], on_update=[])
        ev.debug = mybir.OpDebugInfo(
            op_name=f"splitwait:{inst.name}:{w.ant_name}",
            filename="kernel.py", lineno=1)
        nc.register_instruction(ev)
        return ev

    f = nc.m.functions[0]
    blocks = list(f.blocks)

    # EVENT_SEMAPHORE_RANGE_CLEAR supports at most 8 semaphores per
    # instruction on this walrus; chunk wider ranges.
    for blk in blocks:
        old_insts = blk.instructions
        rewritten = []
        changed = False
        for inst in old_insts:
            if (type(inst).__name__ == "InstISA"
                    and inst.op_name == "EVENT_SEMAPHORE_RANGE_CLEAR"):
                d = dict(inst.ant_dict)
                first, last = d["range_first"], d["range_last"]
                if last - first + 1 > 8:
                    changed = True
                    lo = first
                    while lo <= last:
                        hi = min(lo + 7, last)
                        nb = list(inst.instr)
                        nb[13], nb[14] = lo, hi
                        d2 = dict(d)
                        d2["range_first"], d2["range_last"] = lo, hi
                        ni = mybir.InstISA(
                            name=nc.get_next_instruction_name(),
                            isa_opcode=inst.isa_opcode,
                            engine=inst.engine,
                            instr=nb,
                            op_name=inst.op_name,
                            ins=[], outs=[],
                            ant_dict=d2,
                            verify=inst.verify,
                            ant_isa_is_sequencer_only=inst.ant_isa_is_sequencer_only,
                        )
                        if inst.sync_info is not None and lo == first:
                            ni.sync_info = inst.sync_info
                        nc.register_instruction(ni)
                        rewritten.append(ni)
                        lo = hi + 1
                    continue
            rewritten.append(inst)
        if changed:
            blk.instructions = rewritten

    for bi, blk in enumerate(blocks):
        old = blk.instructions
        if not any(i.sync_info is not None and len(i.sync_info.on_wait) > 1
                   for i in old):
            continue
        new = []
        hoist_prev = []  # evsems that must run before this block is entered
        for idx, inst in enumerate(old):
            si = inst.sync_info
            is_custom = type(inst).__name__ in ("InstReciprocal",)
            if si is not None and is_custom and (si.on_wait or si.on_update):
                # custom-DVE ops lower to fixed-length ISA payloads that
                # cannot carry embedded sync: hoist waits before, updates
                # after (engine is in-order, semantics unchanged).
                for w in si.on_wait:
                    new.append(mk_ev(inst, w))
                posts = list(si.on_update)
                inst.sync_info = mybir.SyncInfo(on_wait=[], on_update=[])
                new.append(inst)
                for u in posts:
                    ev = mybir.InstEventSemaphore(
                        name=nc.get_next_instruction_name())
                    ev.engine = inst.engine
                    ev.sync_info = mybir.SyncInfo(on_wait=[], on_update=[u])
                    ev.debug = mybir.OpDebugInfo(
                        op_name=f"splitupd:{inst.name}",
                        filename="kernel.py", lineno=1)
                    nc.register_instruction(ev)
                    new.append(ev)
                continue
            if si is not None and len(si.on_wait) > 1 and nsofar[0] < split_max:
                nsofar[0] += 1
                waits = list(si.on_wait)
                evs = [mk_ev(inst, w) for w in waits[:-1]]
                if idx == 0 and bi > 0 and type(inst).__name__ == "InstDrain":
                    # barrier-teardown block: walrus rejects extra
                    # instructions before the first drain, so run the waits
                    # at the tail of the previous block instead.
                    hoist_prev.extend(evs)
                else:
                    new.extend(evs)
                inst.sync_info = mybir.SyncInfo(on_wait=[waits[-1]],
                                                on_update=list(si.on_update))
            new.append(inst)
        blk.instructions = new
        if hoist_prev:
            prev = blocks[bi - 1]
            pinsts = prev.instructions
            cut = len(pinsts)
            while cut > 0 and "Branch" in type(pinsts[cut - 1]).__name__:
                cut -= 1
            prev.instructions = pinsts[:cut] + hoist_prev + pinsts[cut:]


def build_nc():
    nc = bass.Bass(trn_type="TRN2", target_bir_lowering=False)

    def din(name, shape):
        return nc.dram_tensor(name, shape, F32, kind="ExternalInput").ap()

    x9 = din("x9", [9 * BC, NP])  # per tile k: 3 row-group replicas of [x0;x1;1] x GB batches
    xz = din("xz", [BC, 2 * N])   # [x0 | x1], b-partition layout
    ident = din("ident", [128, 128])
    lw = {i: din(f"lw{i}", [3, 2 * BC * 128]) for i in (1, 2, 3)}
    qfx = din("qfx", [3, 3 * H + 1])  # [qx2 | qx3 | fx | fc2b-col]
    mv_in = {i: din(f"mv{i}", [128, 2 * BC * BC]) for i in (1, 2, 3)}
    fc2s_in = din("fc2sw", [128, 2])
    out = nc.dram_tensor("out", [BC], F32, kind="ExternalOutput").ap()

    with ExitStack() as ctx:
        tc = ctx.enter_context(tile.TileContext(nc))
        const = ctx.enter_context(tc.tile_pool(name="const", bufs=1))
        lwp = ctx.enter_context(tc.tile_pool(name="lwp", bufs=2))
        cgp = ctx.enter_context(tc.tile_pool(name="cgp", bufs=4))
        tp = ctx.enter_context(tc.tile_pool(name="tp", bufs=3))
        mvp = ctx.enter_context(tc.tile_pool(name="mvp", bufs=2))
        wk = ctx.enter_context(tc.tile_pool(name="wk", bufs=2))
        ep = ctx.enter_context(tc.tile_pool(name="ep", bufs=2, space="PSUM"))
        up = ctx.enter_context(tc.tile_pool(name="up", bufs=1, space="PSUM"))

        mm = nc.tensor.matmul
        act = nc.scalar.activation
        dve = nc.vector
        dma = nc.sync.dma_start      # HWDGE via SP: coords + boundary writes
        wdma = nc.gpsimd.dma_start   # SWDGE via Pool: weights (off SP's FIFO)

        # ---------------- constants / weights (layer-1 critical path first) --
        lwt = {}

        def load_lwt(i, first_eng=None):
            # write partitions {0-2, 32-34, 64-66}: the three PE row-group
            # replicas of the per-pair lhsT table
            t = lwp.tile([96, 2 * BC * 128], F32R, tag="lwt", name=f"lwt{i}")
            for g in range(3):
                eng = first_eng if (g == 0 and first_eng) else wdma
                eng(out=t[32 * g:32 * g + 3, :], in_=lw[i].bitcast(F32R))
            lwt[i] = t

        mvs = {}

        def load_mv(i):
            t = mvp.tile([128, 2 * BC * BC], F32R, tag="mv", name=f"mv{i}")
            for c in range(2):
                wdma(out=t[:, c * BC * BC:(c + 1) * BC * BC],
                    in_=mv_in[i][:, c * BC * BC:(c + 1) * BC * BC].bitcast(F32R))
            mvs[i] = t

        load_lwt(1)

        deferred = []  # emitted after the first coords tile's DMA

        # touch tanh IMMEDIATELY so the ACT table-set load (~2.7us on HW)
        # overlaps the startup DMAs instead of the first real tanh
        warm = wk.tile([1, 1], F32, tag="warm", name="warm")
        dve.memset(warm, 0.0)
        warm2 = wk.tile([1, 1], F32, tag="warm2", name="warm2")
        act(warm2, warm, AF.Tanh)

        def defer_consts():
            ids = const.tile([128, 128], F32, tag="ids", name="ids")
            dma(out=ids, in_=ident)
            qf = const.tile([3, 3 * H + 1], F32, tag="qf", name="qf")
            dma(out=qf, in_=qfx)
            fc2s = const.tile([128, 2], F32, tag="fc2s", name="fc2s")
            dma(out=fc2s, in_=fc2s_in)
            xzs = const.tile([BC, 2 * N], F32, tag="xzs", name="xzs")
            dma(out=xzs, in_=xz)
            zs1 = const.tile([3, BC], F32, tag="zs1", name="zs1")
            dma(out=zs1[2:3, :], in_=x9[2 * GB:2 * GB + 1, 0:BC])  # ones row
            deferred.extend([ids, qf, fc2s, xzs, zs1])

        qxs_sl = {2: (0, H), 3: (H, 2 * H)}
        qsb = [None]   # bias-mode qeff [128, 2*BC] SBUF, set at each boundary
        NBIAS = 4      # slots per layer (li>=2) that run with ACT bias

        cg_tiles = {}

        def get_cg(kh):
            # half-width coords tiles: key = global (tile, n-half) index; 4
            # buffers deep so replica DMAs fire ~3 half-tiles ahead of use
            if kh >= 3 * NG * 2:
                return None
            if kh not in cg_tiles:
                t = cgp.tile([96, GB, 512], F32R, tag="cg", name="cg")
                kk = (kh // 2) % NG
                hs = (kh % 2) * 512
                for a in range(3):
                    s = x9[9 * GB * kk + 3 * GB * a:
                           9 * GB * kk + 3 * GB * (a + 1), hs:hs + 512]
                    # middle replica rides the (idle) SWDGE ring so the three
                    # arrivals overlap instead of serializing on SP's HWDGE
                    eng = wdma if a == 1 else dma
                    eng(out=t[32 * a:32 * a + 3, :, :],
                        in_=s.rearrange("(c g) n -> c g n", c=3).bitcast(F32R))
                cg_tiles[kh] = t
            return cg_tiles[kh]

        get_cg(0)
        get_cg(1)
        load_mv(1)
        get_cg(2)
        get_cg(3)
        defer_consts()
        ids, qf, fc2s, xzs, zs1 = deferred

        # ---------------- layers ----------------
        for li in (1, 2, 3):
            U = up.tile([32, 1024], F32, tag="U", name="U")
            P = wk.tile([BC, N], F32, tag="P", name="P")
            # stats cols: 0 es0, 1 es1, 2 s00, 3 s01, 4 s10, 5 s11,
            #             6 ssum, 7 s0, 8 s1, 9 rinv, 10 z0, 11 z1
            st = wk.tile([BC, 12], F32, tag="st", name="st")
            mvl = mvs[li]
            lwl = lwt[li]


            ucnt = [0, 0]      # u-matmuls emitted per n-half
            pend = []          # [(t_tile, group), ...] deferred u-matmul work
            expq = []          # deferred half-0 softmax emission
            group = []         # (j, cb, gi, h) of current e-tile
            pe_cur = [None]

            def emit_us(t, grp):
                for (j, cb, gi_, h) in grp:
                    hs, hw = HALF[h]
                    cnt = ucnt[h]
                    ucnt[h] += 1
                    mm(U[:, hs:hs + hw], lhsT=mvl[:, cb * BC:(cb + 1) * BC],
                       rhs=t[:, j * 512:j * 512 + hw],
                       start=(cnt == 0), stop=(cnt == 2 * BC - 1))
                    if h == 0 and cnt == 2 * BC - 1:
                        expq.append(2)  # emit half-0 softmax 2 flushes later

            def emit_exp_half(h):
                hs, hw = HALF[h]
                hw = min(hw, N - hs)
                act(P[:, hs:hs + hw], U[:, hs:hs + hw], AF.Exp,
                    accum_out=st[:, 3 * h:3 * h + 1])
                junk = wk.tile([BC, 512], F32, tag="pxs", name="pxs")
                dve.scalar_tensor_tensor(
                    out=junk[:, 0:hw], in0=P[:, hs:hs + hw], scalar=1.0,
                    in1=xzs[:, hs:hs + hw], op0=ALU.mult, op1=ALU.mult,
                    accum_out=st[:, 3 * h + 1:3 * h + 2])
                junk2 = wk.tile([BC, 512], F32, tag="pxs2", name="pxs2")
                dve.scalar_tensor_tensor(
                    out=junk2[:, 0:hw], in0=P[:, hs:hs + hw], scalar=1.0,
                    in1=xzs[:, N + hs:N + hs + hw], op0=ALU.mult, op1=ALU.mult,
                    accum_out=st[:, 3 * h + 2:3 * h + 3])

            def flush(bias=None):
                if not group:
                    return
                g = len(group)
                t = tp.tile([128, GRP * 512], F32R, tag="t", name="t")
                if bias is None:
                    act(t[:, :g * 512], pe_cur[0][:, :g * 512], AF.Tanh)
                else:
                    act(t[:, :g * 512], pe_cur[0][:, :g * 512], AF.Tanh,
                        bias=bias)
                pend.append((t, list(group)))
                group.clear()
                pe_cur[0] = None
                if len(pend) > 2:
                    tt, grp = pend.pop(0)
                    emit_us(tt, grp)
                for i in range(len(expq)):
                    expq[i] -= 1
                if expq and expq[0] <= 0:
                    expq.pop(0)
                    emit_exp_half(0)

            for k in range(NG):
                if li == 1 and k == 1:
                    # prefetch later layers' weights mid-layer-1
                    load_lwt(2)
                    load_mv(2)
                if li == 2 and k == 1:
                    load_lwt(3)
                    load_mv(3)
                for h in range(2):
                    hs, hw = HALF[h]
                    kh = ((li - 1) * NG + k) * 2 + h
                    cg = get_cg(kh)
                    get_cg(kh + 1)  # prefetch (incl. next layer's)
                    get_cg(kh + 2)
                    get_cg(kh + 3)
                    for gi in range(GB):
                        b = k * GB + gi
                        for c in range(2):
                            cb = c * BC + b
                            slot = 32 * k + 16 * h + 2 * gi + c
                            if li > 1 and slot < NBIAS:
                                # bias mode: K=2 (W rows only) + ACT bias;
                                # runs while the qeffT flatten DMAs land
                                pe_cur[0] = ep.tile([128, GRP * 512], F32,
                                                    tag="pe", name="pe")
                                mm(pe_cur[0][:, 0:hw],
                                   lhsT=lwl[0:2, cb * 128:(cb + 1) * 128],
                                   rhs=cg[0:2, gi, 0:hw],
                                   start=True, stop=True)
                                group.append((0, cb, gi, h))
                                flush(bias=qsb[0][:, cb:cb + 1])
                                continue
                            j = len(group)
                            if j == 0:
                                pe_cur[0] = ep.tile([128, GRP * 512], F32,
                                                    tag="pe", name="pe")
                            mm(pe_cur[0][:, j * 512:j * 512 + hw],
                               lhsT=lwl[32 * j:32 * j + 3,
                                        cb * 128:(cb + 1) * 128],
                               rhs=cg[32 * j:32 * j + 3, gi, 0:hw],
                               start=True, stop=True)
                            group.append((j, cb, gi, h))
                            if len(group) == GRP:
                                flush()
            flush()
            while pend:
                tt, grp = pend.pop(0)
                emit_us(tt, grp)
            while expq:
                expq.pop(0)
                emit_exp_half(0)

            # ---- layer end: half-1 softmax, z, next-layer qeff fold ----
            emit_exp_half(1)
            # ssum first, reciprocal EARLY; the s0/s1 sums are emitted after
            # it so the spair muls' same-engine wait count covers the custom
            # op's out-of-band completion (same shape the prior kernel used)
            dve.tensor_tensor(out=st[:, 6:7], in0=st[:, 0:1], in1=st[:, 3:4],
                              op=ALU.add)
            rinv = wk.tile([BC, 1], F32, tag="rinv", name="rinv")
            dve.reciprocal(rinv, st[:, 6:7])
            dve.tensor_tensor(out=st[:, 7:9], in0=st[:, 1:3], in1=st[:, 4:6],
                              op=ALU.add)
            spair = wk.tile([BC, 2], F32, tag="spair", name="spair")
            dve.tensor_scalar(out=spair, in0=st[:, 7:9], scalar1=rinv,
                              scalar2=None, op0=ALU.mult)
            zp = ep.tile([2, BC], F32, tag="pe", name="zp")
            nc.tensor.transpose(zp, spair, ids[0:BC, 0:BC])
            dve.tensor_copy(zs1[0:2, :], zp)

            if li < 3:
                q0, q1 = qxs_sl[li + 1]
                # bias-mode source first: qeff as [h-in-c, cb] -> SBUF, so the
                # next layer's first slots can start before the qeffT flatten
                qp = ep.tile([128, 2 * BC], F32, tag="pe", name="qp")
                for c in range(2):
                    mm(qp[:, c * BC:(c + 1) * BC],
                       lhsT=qf[:, q0 + c * 128:q0 + (c + 1) * 128],
                       rhs=zs1, start=True, stop=True)
                qsb_t = wk.tile([128, 2 * BC], F32, tag="qsb", name="qsb")
                dve.tensor_copy(qsb_t, qp)
                qsb[0] = qsb_t
                # qeffT[cb, h] directly: per c-half, out partitions c*32..+32
                qtp = ep.tile([2 * BC, 128], F32, tag="pe", name="qtp")
                for c in range(2):
                    mm(qtp[c * BC:(c + 1) * BC, :], lhsT=zs1,
                       rhs=qf[:, q0 + c * 128:q0 + (c + 1) * 128],
                       start=True, stop=True)
                qtb = wk.tile([2 * BC, 128], F32, tag="qtb", name="qtb")
                dve.tensor_copy(qtb, qtp)
                for g in range(3):
                    dma(out=lwt[li + 1][32 * g + 2:32 * g + 3, :],
                        in_=qtb.bitcast(F32R))

        # ---------------- head ----------------
        hp = ep.tile([128, 2 * BC], F32, tag="pe", name="hp")
        for c in range(2):
            mm(hp[:, c * BC:(c + 1) * BC],
               lhsT=qf[:, 2 * H + c * 128:2 * H + (c + 1) * 128],
               rhs=zs1, start=True, stop=True)
        r = wk.tile([128, 2 * BC], F32, tag="R", name="R")
        act(r, hp, AF.Relu)
        op = ep.tile([1, BC], F32, tag="pe", name="op")
        for c in range(2):
            mm(op, lhsT=fc2s[:, c:c + 1], rhs=r[:, c * BC:(c + 1) * BC],
               start=(c == 0), stop=(c == 1))
        osb = wk.tile([1, BC], F32, tag="osb", name="osb")
        dve.tensor_scalar_add(osb, op, qf[0:1, 3 * H:3 * H + 1])
        dma(out=out.unsqueeze(0), in_=osb)

    _split_multi_waits(nc)
    return nc


_NC = None


def _get_nc():
    global _NC
    if _NC is None:
        _NC = build_nc()
    return _NC


def make_in_maps(inputs):
    """Host-side fold of all weight algebra + sharding into per-core maps."""
    f = np.float32
    ins = {k: np.ascontiguousarray(np.asarray(v, dtype=f))
           for k, v in inputs.items()}
    static = ins["static"]
    assert static.shape == (B, N, 2)
    enc_w, enc_b = ins["enc_w"], ins["enc_b"]

    W, bb = {}, {}
    for i in (1, 2, 3):
        W[i] = ins[f"ref_w{i}"] @ enc_w                      # [H, 2]
        bb[i] = ins[f"ref_w{i}"] @ enc_b + ins[f"ref_b{i}"]  # [H]

    def build_lw(i, qrow):
        # lw[k, cb*128 + h]: rows 0,1 = W_i[c*128+h, k]; row 2 = qrow[c*128+h]
        arr = np.zeros((3, 2, BC, 128), dtype=f)
        for c in (0, 1):
            blk = W[i][c * 128:(c + 1) * 128, :]             # [128, 2]
            arr[0, c, :, :] = blk[:, 0][None, :]
            arr[1, c, :, :] = blk[:, 1][None, :]
            if qrow is not None:
                arr[2, c, :, :] = qrow[c * 128:(c + 1) * 128][None, :]
        return np.ascontiguousarray(arr.reshape(3, 2 * BC * 128))

    qrow1 = bb[1] + ins["q_b1"]
    lw = {1: build_lw(1, qrow1), 2: build_lw(2, None), 3: build_lw(3, None)}

    qfx = np.zeros((3, 3 * H + 1), dtype=f)
    for i in (2, 3):
        qw = ins[f"q_w{i}"]
        m = np.zeros((3, H), dtype=f)
        m[0:2, :] = (qw @ W[i - 1]).T
        m[2, :] = bb[i] + qw @ bb[i - 1] + ins[f"q_b{i}"]
        qfx[:, (i - 2) * H:(i - 1) * H] = m
    qfx[0:2, 2 * H:3 * H] = (ins["fc1_w"] @ W[3]).T
    qfx[2, 2 * H:3 * H] = ins["fc1_w"] @ bb[3] + ins["fc1_b"]
    qfx[0, 3 * H] = ins["fc2_b"][0]

    mv = {}
    for i in (1, 2, 3):
        v = ins[f"v{i}"]
        m = np.zeros((128, 2, BC, BC), dtype=f)
        for c in (0, 1):
            m[:, c, np.arange(BC), np.arange(BC)] = v[c * 128:(c + 1) * 128, None]
        mv[i] = np.ascontiguousarray(m.reshape(128, 2 * BC * BC))

    fc2s = np.ascontiguousarray(
        ins["fc2_w"].reshape(2, 128).T)               # fc2s[p, j] = w[0, j*128+p]
    ident = np.eye(128, dtype=f)

    shared = {"ident": ident, "qfx": qfx, "fc2sw": fc2s}
    for i in (1, 2, 3):
        shared[f"lw{i}"] = lw[i]
        shared[f"mv{i}"] = mv[i]

    in_maps = []
    for core in range(NCORES):
        sh = static[core * BC:(core + 1) * BC]
        xr = sh.reshape(2 * BC, N)        # raw reshape, matches the reference
        blk = np.zeros((NG, 3, GB, NP), dtype=f)
        blk[:, 0, :, :N] = xr[0::2].reshape(NG, GB, N)
        blk[:, 1, :, :N] = xr[1::2].reshape(NG, GB, N)
        blk[:, 2, :, :N] = 1.0
        x9c = np.ascontiguousarray(
            np.broadcast_to(blk[:, None], (NG, 3, 3, GB, NP))
        ).reshape(9 * BC, NP)
        m = dict(shared)
        m["x9"] = x9c
        xzc = np.empty((BC, 2 * N), dtype=f)
        xzc[:, :N] = xr[0::2]
        xzc[:, N:] = xr[1::2]
        m["xz"] = xzc
        in_maps.append(m)
    return in_maps


def kernel(**inputs) -> np.ndarray:
    nc = _get_nc()
    in_maps = make_in_maps(inputs)
    res = run_bass_kernel_spmd(nc, in_maps, list(range(NCORES)))
    outs = [np.asarray(res.results[c]["out"], dtype=np.float32)
            for c in range(NCORES)]
    return np.concatenate(outs, axis=0).reshape(B, 1)
